# revision 1
# baseline (speedup 1.0000x reference)
"""CrossAttention Trainium2 Bass kernel.

Full inputs in, full output out. Data-parallel over batch: 8 batch elements
-> 8 NeuronCores; each core runs the whole cross-attention for one batch
element. Weights are replicated; no collectives.

Per-core computation (transposed domain end-to-end):
  x [512, 4096] (c-major)  -> qT = Wq.T @ x            [512(i), 4096(t)]
  ctx [77, 768]            -> k/v = ctxT.T @ Wk/Wv     [77(j), 512(i)]
  per head h (d=64):  simT = kT_h.T @ qT_h             [77(j), t]
                      expsim = exp(simT / 8)           (scale fused in ACT)
                      av = [v_h | 1].T @ expsim        [65, t] (row 64 = rowsum)
                      outUT_h = av[0:64] * recip(rowsum)  (bcast via DRAM)
  y = Wo.T @ outUT + bo                                [512(c), 4096(t)]

All matmuls run in float32r (fp32 read as fp22, 1 PE cycle/row at N=512).
"""

import os
import sys

for _p in ("/opt/trn_rl_repo", "/root/.axon_site/_ro/trn_rl_repo"):
    if os.path.isdir(_p) and _p not in sys.path:
        sys.path.insert(0, _p)

import numpy as np

C = 512        # channels / model dim
T = 4096       # tokens (H*W)
S = 77         # context length
DCTX = 768     # context dim
HEADS = 8
DH = 64        # head dim
NT = 8         # token chunks
TC = T // NT   # 512 tokens per chunk
CT = C // 128  # 4 c-tiles
KT = DCTX // 128  # 6 context-dim tiles

# how many of the 8 per-chunk normalize-multiplies run on GPSIMD (rest DVE)
N_NORM_GPSIMD = 4

_BUILT = None


def _build(dbg=False):
    import concourse.mybir as mybir
    import concourse.tile as tile
    from concourse import bacc
    from concourse.masks import make_identity

    f32 = mybir.dt.float32
    f32r = mybir.dt.float32r
    AF = mybir.ActivationFunctionType

    nc = bacc.Bacc("TRN2", target_bir_lowering=False, debug=False, num_devices=8)

    X = nc.dram_tensor("x", [C, T], f32, kind="ExternalInput")
    CTX = nc.dram_tensor("ctx", [S, DCTX], f32, kind="ExternalInput")
    WQ = nc.dram_tensor("wq", [C, C], f32, kind="ExternalInput")
    WK = nc.dram_tensor("wk", [DCTX, C], f32, kind="ExternalInput")
    WV = nc.dram_tensor("wv", [DCTX, C], f32, kind="ExternalInput")
    WO = nc.dram_tensor("wo", [C, C], f32, kind="ExternalInput")
    BO = nc.dram_tensor("bo", [C], f32, kind="ExternalInput")
    Y = nc.dram_tensor("y", [C, T], f32, kind="ExternalOutput")
    if dbg:
        DQ = nc.dram_tensor("dq", [128, CT, TC], f32, kind="ExternalOutput")
        DE = nc.dram_tensor("de", [S, TC], f32, kind="ExternalOutput")
        DAVT = nc.dram_tensor("davt", [DH + 1, TC], f32, kind="ExternalOutput")
        DRSC = nc.dram_tensor("drsc", [64, TC // 8], f32, kind="ExternalOutput")
        DRSR = nc.dram_tensor("drsr", [64, TC // 8], f32, kind="ExternalOutput")
        DBC = nc.dram_tensor("dbc", [64, HEADS, TC], f32, kind="ExternalOutput")
        DOU = nc.dram_tensor("dou", [128, CT, TC], f32, kind="ExternalOutput")
        DKT = nc.dram_tensor("dkt", [128, HEADS // 2, S], f32, kind="ExternalOutput")
        DVO = nc.dram_tensor("dvo", [S, HEADS, DH + 1], f32, kind="ExternalOutput")

    with tile.TileContext(nc) as tc:
        with (
            tc.tile_pool(name="static", bufs=1) as st,
            tc.tile_pool(name="xin", bufs=3) as xp,
            tc.tile_pool(name="qt", bufs=2) as qp,
            tc.tile_pool(name="expsim", bufs=6) as ep,
            tc.tile_pool(name="avs", bufs=12) as ap_,
            tc.tile_pool(name="outut", bufs=2) as op_,
            tc.tile_pool(name="bcast", bufs=2) as bp,
            tc.tile_pool(name="ysb", bufs=4) as yp,
            tc.tile_pool(name="small", bufs=3) as sp,
            tc.tile_pool(name="dram", bufs=2, space="DRAM") as dp,
        ):
            # ---- static loads (ctx/wq first: earliest consumers) --------------
            ctxs = st.tile([S, DCTX], f32, tag="ctxs")
            nc.sync.dma_start(ctxs[:], CTX[:])
            ident = st.tile([128, 128], f32, tag="ident")
            make_identity(nc, ident[:])
            wq = st.tile([128, CT, C], f32r, tag="wq")
            nc.sync.dma_start(wq[:], WQ[:].bitcast(f32r).rearrange("(o p) i -> p o i", p=128))
            wk = st.tile([128, KT, C], f32r, tag="wk")
            nc.sync.dma_start(wk[:], WK[:].bitcast(f32r).rearrange("(o p) i -> p o i", p=128))
            wv = st.tile([128, KT, C], f32r, tag="wv")
            nc.sync.dma_start(wv[:], WV[:].bitcast(f32r).rearrange("(o p) i -> p o i", p=128))
            wo = st.tile([128, CT, C], f32r, tag="wo")
            nc.sync.dma_start(wo[:], WO[:].bitcast(f32r).rearrange("(o p) c -> p o c", p=128))
            bo = st.tile([128, CT], f32, tag="bo")
            nc.sync.dma_start(bo[:], BO[:].rearrange("(o p) -> p o", p=128))

            # ---- setup: context transpose, K/V projections --------------------
            ctxT = st.tile([128, KT, S], f32r, tag="ctxT")
            ktp = st.tile([128, HEADS // 2, S], f32r, tag="ktp")  # kT head-pairs
            vone = st.tile([S, HEADS, DH + 1], f32r, tag="vone")  # [v_h | ones]
            with tc.tile_pool(name="ps_setup", bufs=1, space="PSUM") as ps_st:
                for ct in range(KT):
                    tp = ps_st.tile([128, S], f32, tag=f"ctx_t{ct % 2}")
                    nc.tensor.transpose(tp[:], ctxs[:, ct * 128:(ct + 1) * 128], ident[0:S, 0:S])
                    nc.vector.tensor_copy(ctxT[:, ct, :], tp[:])
                kps = ps_st.tile([S, C], f32, tag="kproj")
                vps = ps_st.tile([S, C], f32, tag="vproj")
                for ct in range(KT):
                    nc.tensor.matmul(kps[:], ctxT[:, ct, :], wk[:, ct, :],
                                     start=(ct == 0), stop=(ct == KT - 1))
                for ct in range(KT):
                    nc.tensor.matmul(vps[:], ctxT[:, ct, :], wv[:, ct, :],
                                     start=(ct == 0), stop=(ct == KT - 1))
                ksb = st.tile([S, C], f32, tag="ksb")
                nc.vector.tensor_copy(ksb[:], kps[:])
                for h in range(HEADS):
                    nc.vector.tensor_copy(vone[:, h, 0:DH], vps[:, h * DH:(h + 1) * DH].bitcast(f32r))
                    nc.vector.memset(vone[:, h, DH:DH + 1].bitcast(f32), 1.0)
                for h in range(HEADS):
                    tp = ps_st.tile([DH, S], f32, tag=f"k_t{h % 2}")
                    nc.tensor.transpose(tp[:], ksb[:, h * DH:(h + 1) * DH], ident[0:S, 0:S])
                    base = (h % 2) * DH
                    nc.vector.tensor_copy(ktp[base:base + DH, h // 2, :], tp[:])

            with (
                tc.tile_pool(name="ps_q", bufs=2, space="PSUM") as ps_q,
                tc.tile_pool(name="ps_sim", bufs=2, space="PSUM") as ps_sim,
                tc.tile_pool(name="ps_av", bufs=2, space="PSUM") as ps_av,
                tc.tile_pool(name="ps_y", bufs=2, space="PSUM") as ps_y,
            ):
                # ---- main loop over token chunks -----------------------------
                def oproj_group(t, ou, ct):
                    tsl = slice(t * TC, (t + 1) * TC)
                    py = ps_y.tile([128, TC], f32, tag="py")
                    for it in range(CT):
                        nc.tensor.matmul(py[:], wo[:, it, ct * 128:(ct + 1) * 128], ou[:, it, :],
                                         start=(it == 0), stop=(it == CT - 1))
                    ys = yp.tile([128, TC], f32, tag="ys")
                    if ct % 2 == 0:
                        nc.scalar.activation(ys[:], py[:], AF.Identity, bias=bo[:, ct:ct + 1])
                    else:
                        nc.vector.tensor_scalar_add(ys[:], py[:], bo[:, ct:ct + 1])
                    nc.sync.dma_start(
                        Y[:].rearrange("(o p) t -> p o t", p=128)[:, ct, tsl], ys[:])

                def oproj(t, ou):
                    for ct in range(CT):
                        oproj_group(t, ou, ct)

                prev = None
                for t in range(NT):
                    tsl = slice(t * TC, (t + 1) * TC)
                    xs = xp.tile([128, CT, TC], f32r, tag="xs")
                    nc.sync.dma_start(
                        xs[:], X[:].bitcast(f32r).rearrange("(o p) t -> p o t", p=128)[:, :, tsl])

                    # Q projection -> qT [128, 4, TC] (i on partitions)
                    qt = qp.tile([128, CT, TC], f32r, tag="qt")
                    for it in range(CT):
                        pq = ps_q.tile([128, TC], f32, tag="pq")
                        for ct in range(CT):
                            nc.tensor.matmul(pq[:], wq[:, ct, it * 128:(it + 1) * 128], xs[:, ct, :],
                                             start=(ct == 0), stop=(ct == CT - 1))
                        nc.vector.tensor_copy(qt[:, it, :], pq[:])

                    if dbg and t == 0:
                        nc.sync.dma_start(DQ[:], qt[:].bitcast(f32))
                        nc.sync.dma_start(DKT[:], ktp[:].bitcast(f32))
                        nc.sync.dma_start(DVO[:], vone[:].bitcast(f32))
                    # QK^T per head + exp (scale 1/8 fused in ACT)
                    exps = []
                    for h in range(HEADS):
                        base = (h % 2) * DH
                        psim = ps_sim.tile([S, TC], f32, tag="psim")
                        nc.tensor.matmul(psim[:], ktp[base:base + DH, h // 2, :],
                                         qt[base:base + DH, h // 2, :])
                        es = ep.tile([S, TC], f32r, tag="exps")
                        nc.scalar.activation(es[:], psim[:], AF.Exp, scale=DH ** -0.5)
                        exps.append(es)
                        if dbg and t == 0 and h == 0:
                            nc.sync.dma_start(DE[:], es[:].bitcast(f32))

                    # normalize chunk t-1 (its bcast DMA was issued last
                    # chunk, so the DRAM round-trip latency is fully hidden)
                    if prev is not None:
                        p_avts, p_bc = prev
                        ou = op_.tile([128, CT, TC], f32r, tag="ou")
                        for h in range(HEADS):
                            base = (h % 2) * DH
                            eng = nc.gpsimd if h < N_NORM_GPSIMD else nc.vector
                            eng.tensor_tensor(
                                ou[base:base + DH, h // 2, :], p_avts[h][0:DH, :],
                                p_bc[:, h, :], mybir.AluOpType.mult)
                        if dbg and t == 1:
                            nc.sync.dma_start(DOU[:], ou[:].bitcast(f32))
                    else:
                        ou = None

                    # AV (+ rowsum via ones column), evac split ACT/DVE,
                    # interleaved with chunk t-1's output projection so the
                    # in-order PE always has independent matmul work.
                    rraw = dp.tile([HEADS, TC], f32, tag="rraw")
                    rcp = dp.tile([64, TC // 8], f32, tag="rcp")
                    avts = []
                    for h in range(HEADS):
                        pav = ps_av.tile([DH + 1, TC], f32, tag="pav")
                        nc.tensor.matmul(pav[:], vone[:, h, :], exps[h][:])
                        avt = ap_.tile([DH + 1, TC], f32, tag="avt")
                        if h % 2 == 0:
                            nc.scalar.activation(avt[:], pav[:], AF.Copy)
                        else:
                            nc.vector.tensor_copy(avt[:], pav[:])
                        avts.append(avt)
                        if dbg and t == 0 and h == 0:
                            nc.sync.dma_start(DAVT[:], avt[:])
                        nc.sync.dma_start(rraw[h, None, :], avt[DH:DH + 1, :])
                        if h % 2 == 1 and ou is not None:
                            oproj_group(t - 1, ou, h // 2)

                    # compact reciprocal of rowsums, bounce through DRAM,
                    # issue the broadcast now; consumed next chunk.
                    rsc = sp.tile([64, TC // 8], f32, tag="rsc")
                    nc.sync.dma_start(rsc[:], rraw[:].rearrange("h t -> (h t)").rearrange("(a b) -> a b", a=64))
                    rsr = sp.tile([64, TC // 8], f32, tag="rsr")
                    nc.vector.reciprocal(rsr[:], rsc[:])
                    if dbg and t == 0:
                        nc.sync.dma_start(DRSC[:], rsc[:])
                        nc.sync.dma_start(DRSR[:], rsr[:])
                    nc.sync.dma_start(rcp[:], rsr[:])
                    bc = bp.tile([64, HEADS, TC], f32, tag="bc")
                    nc.sync.dma_start(
                        bc[:].rearrange("p h t -> p (h t)"),
                        rcp[:].rearrange("a b -> (a b)")[None, :]
                        .to_broadcast((64, HEADS * TC)))
                    if dbg and t == 0:
                        nc.sync.dma_start(DBC[:], bc[:])

                    prev = (avts, bc)

                # drain: normalize + project the last chunk
                p_avts, p_bc = prev
                ou = op_.tile([128, CT, TC], f32r, tag="ou")
                for h in range(HEADS):
                    base = (h % 2) * DH
                    eng = nc.gpsimd if h < N_NORM_GPSIMD else nc.vector
                    eng.tensor_tensor(
                        ou[base:base + DH, h // 2, :], p_avts[h][0:DH, :],
                        p_bc[:, h, :], mybir.AluOpType.mult)
                oproj(NT - 1, ou)

    nc.compile()
    return nc


def _get_nc():
    global _BUILT
    if _BUILT is None:
        _BUILT = _build()
    return _BUILT


def kernel(x, context, Wq, Wk, Wv, Wo, bo):
    from concourse.bass_utils import run_bass_kernel_spmd

    B = x.shape[0]
    assert B == 8 and x.shape == (8, C, 64, 64)
    nc = _get_nc()
    x = np.ascontiguousarray(np.asarray(x, dtype=np.float32))
    in_maps = [
        {
            "x": x[b].reshape(C, T),
            "ctx": np.ascontiguousarray(np.asarray(context[b], np.float32)),
            "wq": np.asarray(Wq, np.float32),
            "wk": np.asarray(Wk, np.float32),
            "wv": np.asarray(Wv, np.float32),
            "wo": np.asarray(Wo, np.float32),
            "bo": np.asarray(bo, np.float32),
        }
        for b in range(B)
    ]
    res = run_bass_kernel_spmd(nc, in_maps, core_ids=list(range(8)))
    return np.stack([r["y"].reshape(C, 64, 64) for r in res.results]).astype(np.float32)



# revision 16
# speedup vs baseline: 1.6313x; 1.6313x over previous
"""CrossAttention Trainium2 Bass kernel.

Full inputs in, full output out. Data-parallel over batch: 8 batch elements
-> 8 NeuronCores; each core runs the whole cross-attention for one batch
element. Weights are replicated; no collectives.

Per-core computation (transposed domain end-to-end):
  x [512, 4096] (c-major)  -> qT = Wq.T @ x            [512(i), 4096(t)]
  ctx [77, 768]            -> k/v = ctxT.T @ Wk/Wv     [77(j), 512(i)]
  per head h (d=64):  simT = kT_h.T @ qT_h             [77(j), t]
                      es = exp(simT / 8)               (scale fused in ACT)
  rowsums: rs8[h,t] = sum_j es_h[j,t] via 8 accumulating selector matmuls
           into one [8, TC] PSUM tile; recip8 = 1/rs8 (DVE approx, 1 op)
  bc_p [128, TC] = selector matmul broadcast of recip8 rows (2p, 2p+1)
  AV pairs col-tiled: pav_p [128, TC] = [v_2p|v_2p+1].T-style pair bank
  ou[:, p, :] = pav_p * bc_p  (single fused DVE tensor_tensor per pair,
           PSUM x PSUM -> SBUF; this is both the PSUM evac and softmax div)
  y = Wo.T @ ou + bo                                   [512(c), 4096(t)]

All matmuls run in float32r (fp32 read as fp22, 1 PE cycle/row at N=512).
No DRAM round trips for the softmax denominator: rowsum packing and the
partition-broadcast both run on the PE via tiny static selector weights.
"""

import os
import sys

for _p in ("/opt/trn_rl_repo", "/root/.axon_site/_ro/trn_rl_repo"):
    if os.path.isdir(_p) and _p not in sys.path:
        sys.path.insert(0, _p)

import numpy as np

C = 512        # channels / model dim
T = 4096       # tokens (H*W)
S = 77         # context length
DCTX = 768     # context dim
HEADS = 8
DH = 64        # head dim
NT = 8         # token chunks
TC = T // NT   # 512 tokens per chunk
CT = C // 128  # 4 c-tiles
KT = DCTX // 128  # 6 context-dim tiles
NP = HEADS // 2   # 4 head pairs

_BUILT = None


def _build(dbg=False):
    import concourse.mybir as mybir
    import concourse.tile as tile
    from concourse import bacc
    from concourse.masks import make_identity

    f32 = mybir.dt.float32
    f32r = mybir.dt.float32r
    AF = mybir.ActivationFunctionType

    nc = bacc.Bacc("TRN2", target_bir_lowering=False, debug=False, num_devices=8)

    X = nc.dram_tensor("x", [C, T], f32, kind="ExternalInput")
    CTX = nc.dram_tensor("ctx", [S, DCTX], f32, kind="ExternalInput")
    WQ = nc.dram_tensor("wq", [C, C], f32, kind="ExternalInput")
    WK = nc.dram_tensor("wk", [DCTX, C], f32, kind="ExternalInput")
    WV = nc.dram_tensor("wv", [DCTX, C], f32, kind="ExternalInput")
    WO = nc.dram_tensor("wo", [C, C], f32, kind="ExternalInput")
    BO = nc.dram_tensor("bo", [C], f32, kind="ExternalInput")
    Y = nc.dram_tensor("y", [C, T], f32, kind="ExternalOutput")
    if dbg:
        DQ = nc.dram_tensor("dq", [128, CT, TC], f32, kind="ExternalOutput")
        DE = nc.dram_tensor("de", [S, TC], f32, kind="ExternalOutput")
        DRS = nc.dram_tensor("drs", [HEADS, TC], f32, kind="ExternalOutput")
        DRC = nc.dram_tensor("drc", [HEADS, TC], f32, kind="ExternalOutput")
        DBC = nc.dram_tensor("dbc", [128, TC], f32, kind="ExternalOutput")
        DOU = nc.dram_tensor("dou", [128, CT, TC], f32, kind="ExternalOutput")

    with tile.TileContext(nc) as tc:
        with (
            tc.tile_pool(name="static", bufs=1) as st,
            tc.tile_pool(name="xin", bufs=3) as xp,
            tc.tile_pool(name="qt", bufs=2) as qp,
            tc.tile_pool(name="expsim", bufs=6) as ep,
            tc.tile_pool(name="outut", bufs=2) as op_,
            tc.tile_pool(name="rcp", bufs=2) as rp,
            tc.tile_pool(name="bcast", bufs=3) as bp,
            tc.tile_pool(name="ysb", bufs=4) as yp,
        ):
            # ---- static loads (ctx/wq first: earliest consumers) --------------
            ctxs = st.tile([S, DCTX], f32, tag="ctxs")
            nc.sync.dma_start(ctxs[:], CTX[:])
            ident = st.tile([128, 128], f32, tag="ident")
            make_identity(nc, ident[:])
            wq = st.tile([128, CT, C], f32r, tag="wq")
            nc.sync.dma_start(wq[:], WQ[:].bitcast(f32r).rearrange("(o p) i -> p o i", p=128))
            wk = st.tile([128, KT, C], f32r, tag="wk")
            nc.sync.dma_start(wk[:], WK[:].bitcast(f32r).rearrange("(o p) i -> p o i", p=128))
            wv = st.tile([128, KT, C], f32r, tag="wv")
            nc.sync.dma_start(wv[:], WV[:].bitcast(f32r).rearrange("(o p) i -> p o i", p=128))
            wo = st.tile([128, CT, C], f32r, tag="wo")
            nc.sync.dma_start(wo[:], WO[:].bitcast(f32r).rearrange("(o p) c -> p o c", p=128))
            bo = st.tile([128, CT], f32, tag="bo")
            nc.sync.dma_start(bo[:], BO[:].rearrange("(o p) -> p o", p=128))

            # selector weights (static), built with affine_select (iota
            # compare, out = compare(iota, 0) ? in_ : fill) in f32 scratch,
            # then CAST into f32r (matmul requires f32r-rounded producers).
            # sel77[j, h, c] = (c == h): rowsum of es_h lands on psum row h
            sel77f = st.tile([S, HEADS, HEADS], f32, tag="sel77f")
            nc.gpsimd.memset(sel77f[:], 0.0)
            nc.gpsimd.affine_select(
                out=sel77f[:], in_=sel77f[:],
                compare_op=mybir.AluOpType.not_equal, fill=1.0,
                base=0, channel_multiplier=0,
                pattern=[[-1, HEADS], [1, HEADS]])  # iota = c - h
            sel77 = st.tile([S, HEADS, HEADS], f32r, tag="sel77")
            nc.vector.tensor_copy(sel77[:], sel77f[:])
            # selbc[j, p, half, c] = (j == 2p + half): bc_p = bcast of rcp8 rows
            selbcf = st.tile([HEADS, NP, 2, DH], f32, tag="selbcf")
            nc.gpsimd.memset(selbcf[:], 0.0)
            nc.gpsimd.affine_select(
                out=selbcf[:], in_=selbcf[:],
                compare_op=mybir.AluOpType.not_equal, fill=1.0,
                base=0, channel_multiplier=1,
                pattern=[[-2, NP], [-1, 2], [0, DH]])  # iota = j - 2p - half
            selbc = st.tile([HEADS, NP, 2, DH], f32r, tag="selbc")
            nc.vector.tensor_copy(selbc[:], selbcf[:])

            # ---- setup: context transpose, K/V projections --------------------
            ctxT = st.tile([128, KT, S], f32r, tag="ctxT")
            ktp = st.tile([128, NP, S], f32r, tag="ktp")    # kT head-pairs
            # vpair[:, p, 0] = [v_2p | 0], vpair[:, p, 1] = [0 | v_2p+1]:
            # zero-padded M=128 stationaries so the AV pair accumulates into
            # one [128, TC] bank without col-tiling (quadrant 3 is invalid).
            vpair = st.tile([S, NP, 2, 128], f32r, tag="vpair")
            nc.gpsimd.memset(vpair[:].bitcast(f32), 0.0)
            with tc.tile_pool(name="ps_setup", bufs=1, space="PSUM") as ps_st:
                for ct in range(KT):
                    tp = ps_st.tile([128, S], f32, tag=f"ctx_t{ct % 2}")
                    nc.tensor.transpose(tp[:], ctxs[:, ct * 128:(ct + 1) * 128], ident[0:S, 0:S])
                    nc.vector.tensor_copy(ctxT[:, ct, :], tp[:])
                kps = ps_st.tile([S, C], f32, tag="kproj")
                vps = ps_st.tile([S, C], f32, tag="vproj")
                for ct in range(KT):
                    nc.tensor.matmul(kps[:], ctxT[:, ct, :], wk[:, ct, :],
                                     start=(ct == 0), stop=(ct == KT - 1))
                for ct in range(KT):
                    nc.tensor.matmul(vps[:], ctxT[:, ct, :], wv[:, ct, :],
                                     start=(ct == 0), stop=(ct == KT - 1))
                ksb = st.tile([S, C], f32, tag="ksb")
                nc.vector.tensor_copy(ksb[:], kps[:])
                for h in range(HEADS):
                    half = h % 2
                    nc.vector.tensor_copy(
                        vpair[:, h // 2, half, half * DH:half * DH + DH],
                        vps[:, h * DH:(h + 1) * DH])
                for h in range(HEADS):
                    tp = ps_st.tile([DH, S], f32, tag=f"k_t{h % 2}")
                    nc.tensor.transpose(tp[:], ksb[:, h * DH:(h + 1) * DH], ident[0:S, 0:S])
                    base = (h % 2) * DH
                    nc.vector.tensor_copy(ktp[base:base + DH, h // 2, :], tp[:])

            with (
                tc.tile_pool(name="ps_gemm", bufs=3, space="PSUM") as ps_g,
                tc.tile_pool(name="ps_sim", bufs=2, space="PSUM") as ps_sim,
                tc.tile_pool(name="ps_av", bufs=2, space="PSUM") as ps_av,
                tc.tile_pool(name="ps_rs", bufs=1, space="PSUM") as ps_rs,
            ):
                # ---- main loop over token chunks -----------------------------
                def oproj_group(t, ou, ct):
                    tsl = slice(t * TC, (t + 1) * TC)
                    py = ps_g.tile([128, TC], f32, tag="pg")
                    for it in range(CT):
                        nc.tensor.matmul(py[:], wo[:, it, ct * 128:(ct + 1) * 128], ou[:, it, :],
                                         start=(it == 0), stop=(it == CT - 1))
                    ys = yp.tile([128, TC], f32, tag="ys")
                    if ct % 2 == 0:
                        nc.scalar.activation(ys[:], py[:], AF.Identity, bias=bo[:, ct:ct + 1])
                    else:
                        nc.vector.tensor_scalar_add(ys[:], py[:], bo[:, ct:ct + 1])
                    nc.sync.dma_start(
                        Y[:].rearrange("(o p) t -> p o t", p=128)[:, ct, tsl], ys[:])

                prev = None
                for t in range(NT):
                    xs = xp.tile([128, CT, TC], f32r, tag="xs")
                    nc.sync.dma_start(
                        xs[:], X[:].bitcast(f32r).rearrange("(o p) t -> p o t", p=128)
                        [:, :, t * TC:(t + 1) * TC])

                    # Q projection -> qT [128, 4, TC] (i on partitions)
                    qt = qp.tile([128, CT, TC], f32r, tag="qt")
                    for it in range(CT):
                        pq = ps_g.tile([128, TC], f32, tag="pg")
                        for ct in range(CT):
                            nc.tensor.matmul(pq[:], wq[:, ct, it * 128:(it + 1) * 128], xs[:, ct, :],
                                             start=(ct == 0), stop=(ct == CT - 1))
                        nc.vector.tensor_copy(qt[:, it, :], pq[:])

                    if dbg and t == 0:
                        nc.sync.dma_start(DQ[:], qt[:].bitcast(f32))

                    # QK^T per head + exp (scale 1/8 fused in ACT); rowsums
                    # accumulate into one [8, TC] psum bank via sel77; AV pairs
                    # col-tiled into one [128, TC] bank; chunk t-1's O
                    # projection groups interleave as PE filler.
                    rs8 = ps_rs.tile([HEADS, TC], f32, tag="rs8")
                    exps = []
                    avps = []
                    ogroups = list(range(CT)) if prev is not None else []

                    def emit_oproj_filler():
                        if ogroups:
                            oproj_group(t - 1, prev[0], ogroups.pop(0))

                    for h in range(HEADS):
                        base = (h % 2) * DH
                        psim = ps_sim.tile([128, TC], f32, tag="psim")
                        nc.tensor.matmul(psim[0:S, :], ktp[base:base + DH, h // 2, :],
                                         qt[base:base + DH, h // 2, :])
                        es = ep.tile([S, TC], f32r, tag="exps")
                        nc.scalar.activation(es[:], psim[0:S, :], AF.Exp, scale=DH ** -0.5)
                        exps.append(es)
                        if dbg and t == 0 and h == 0:
                            nc.sync.dma_start(DE[:], es[:].bitcast(f32))
                        # rowsum accumulate
                        nc.tensor.matmul(rs8[:], sel77[:, h, :],
                                         es[:], start=(h == 0), stop=(h == HEADS - 1))
                        # AV pair: two heads accumulate into one psum bank via
                        # zero-padded M=128 stationaries
                        if h % 2 == 0:
                            pav = ps_av.tile([128, TC], f32, tag="pav")
                            avps.append(pav)
                        nc.tensor.matmul(pav[:], vpair[:, h // 2, h % 2, :],
                                         exps[h][:], start=(h % 2 == 0),
                                         stop=(h % 2 == 1))
                        if h % 2 == 1:
                            emit_oproj_filler()

                    # recip of packed rowsums (single DVE op, ~18-bit accurate)
                    rcp8f = rp.tile([HEADS, TC], f32, tag="rcp8f")
                    nc.vector.reciprocal_approx_fast(rcp8f[:], rs8[:])
                    rcp8 = rp.tile([HEADS, TC], f32r, tag="rcp8")
                    nc.vector.tensor_copy(rcp8[:], rcp8f[:])
                    if dbg and t == 0:
                        nc.sync.dma_start(DRS[:], rs8[:])
                        nc.sync.dma_start(DRC[:], rcp8[:])

                    # broadcast + fused evac/normalize per pair:
                    # bc_p[c,t] = rcp8[2p + c//64, t]; ou[:,p,:] = pav_p * bc_p
                    ou = op_.tile([128, CT, TC], f32r, tag="ou")
                    for p in range(NP):
                        pbc = ps_sim.tile([128, TC], f32, tag="psim")
                        nc.tensor.matmul(pbc[:], selbc[:, p, :, :],
                                         rcp8[:].bitcast(f32r))
                        bcs = bp.tile([128, TC], f32, tag="bcs")
                        nc.scalar.activation(bcs[:], pbc[:], AF.Copy)
                        nc.vector.tensor_tensor(
                            ou[:, p, :], avps[p][:], bcs[:],
                            mybir.AluOpType.mult)
                        if dbg and t == 0 and p == 0:
                            nc.sync.dma_start(DBC[:], bcs[:])
                    if dbg and t == 0:
                        nc.sync.dma_start(DOU[:], ou[:].bitcast(f32))

                    # leftover O-projection groups for chunk t-1
                    while ogroups:
                        emit_oproj_filler()

                    prev = (ou,)

                # drain: O projection of the last chunk
                for ct in range(CT):
                    oproj_group(NT - 1, prev[0], ct)

    nc.compile()
    return nc


def _get_nc():
    global _BUILT
    if _BUILT is None:
        _BUILT = _build()
    return _BUILT


def kernel(x, context, Wq, Wk, Wv, Wo, bo):
    from concourse.bass_utils import run_bass_kernel_spmd

    B = x.shape[0]
    assert B == 8 and x.shape == (8, C, 64, 64)
    nc = _get_nc()
    x = np.ascontiguousarray(np.asarray(x, dtype=np.float32))
    in_maps = [
        {
            "x": x[b].reshape(C, T),
            "ctx": np.ascontiguousarray(np.asarray(context[b], np.float32)),
            "wq": np.asarray(Wq, np.float32),
            "wk": np.asarray(Wk, np.float32),
            "wv": np.asarray(Wv, np.float32),
            "wo": np.asarray(Wo, np.float32),
            "bo": np.asarray(bo, np.float32),
        }
        for b in range(B)
    ]
    res = run_bass_kernel_spmd(nc, in_maps, core_ids=list(range(8)))
    return np.stack([r["y"].reshape(C, 64, 64) for r in res.results]).astype(np.float32)


# revision 19
# speedup vs baseline: 1.7157x; 1.0518x over previous
"""CrossAttention Trainium2 Bass kernel.

Full inputs in, full output out. Data-parallel over batch: 8 batch elements
-> 8 NeuronCores; each core runs the whole cross-attention for one batch
element. Weights are replicated; no collectives.

Per-core computation (transposed domain end-to-end):
  x [512, 4096] (c-major)  -> qT = Wq.T @ x            [512(i), 4096(t)]
  ctx [77, 768]            -> k/v = ctxT.T @ Wk/Wv     [77(j), 512(i)]
  per head h (d=64):  simT = kT_h.T @ qT_h             [77(j), t]
                      es = exp(simT / 8)               (scale fused in ACT)
  rowsums: rs8[h,t] = sum_j es_h[j,t] via 8 accumulating selector matmuls
           into one [8, TC] PSUM tile; recip8 = 1/rs8 (DVE approx, 1 op)
  bc_p [128, TC] = selector matmul broadcast of recip8 rows (2p, 2p+1)
  AV pairs col-tiled: pav_p [128, TC] = [v_2p|v_2p+1].T-style pair bank
  ou[:, p, :] = pav_p * bc_p  (single fused DVE tensor_tensor per pair,
           PSUM x PSUM -> SBUF; this is both the PSUM evac and softmax div)
  y = Wo.T @ ou + bo                                   [512(c), 4096(t)]

All matmuls run in float32r (fp32 read as fp22, 1 PE cycle/row at N=512).
No DRAM round trips for the softmax denominator: rowsum packing and the
partition-broadcast both run on the PE via tiny static selector weights.
"""

import os
import sys

for _p in ("/opt/trn_rl_repo", "/root/.axon_site/_ro/trn_rl_repo"):
    if os.path.isdir(_p) and _p not in sys.path:
        sys.path.insert(0, _p)

import numpy as np

C = 512        # channels / model dim
T = 4096       # tokens (H*W)
S = 77         # context length
DCTX = 768     # context dim
HEADS = 8
DH = 64        # head dim
NT = 8         # token chunks
TC = T // NT   # 512 tokens per chunk
CT = C // 128  # 4 c-tiles
KT = DCTX // 128  # 6 context-dim tiles
NP = HEADS // 2   # 4 head pairs

_BUILT = None


def _build(dbg=False):
    import concourse.mybir as mybir
    import concourse.tile as tile
    from concourse import bacc
    from concourse.masks import make_identity

    f32 = mybir.dt.float32
    f32r = mybir.dt.float32r
    AF = mybir.ActivationFunctionType

    nc = bacc.Bacc("TRN2", target_bir_lowering=False, debug=False, num_devices=8)

    X = nc.dram_tensor("x", [C, T], f32, kind="ExternalInput")
    CTX = nc.dram_tensor("ctx", [S, DCTX], f32, kind="ExternalInput")
    WQ = nc.dram_tensor("wq", [C, C], f32, kind="ExternalInput")
    WK = nc.dram_tensor("wk", [DCTX, C], f32, kind="ExternalInput")
    WV = nc.dram_tensor("wv", [DCTX, C], f32, kind="ExternalInput")
    WO = nc.dram_tensor("wo", [C, C], f32, kind="ExternalInput")
    BO = nc.dram_tensor("bo", [C], f32, kind="ExternalInput")
    Y = nc.dram_tensor("y", [C, T], f32, kind="ExternalOutput")
    if dbg:
        DQ = nc.dram_tensor("dq", [128, CT, TC], f32, kind="ExternalOutput")
        DE = nc.dram_tensor("de", [S, TC], f32, kind="ExternalOutput")
        DRS = nc.dram_tensor("drs", [HEADS, TC], f32, kind="ExternalOutput")
        DRC = nc.dram_tensor("drc", [HEADS, TC], f32, kind="ExternalOutput")
        DBC = nc.dram_tensor("dbc", [128, TC], f32, kind="ExternalOutput")
        DOU = nc.dram_tensor("dou", [128, CT, TC], f32, kind="ExternalOutput")

    with tile.TileContext(nc) as tc:
        with (
            tc.tile_pool(name="static", bufs=1) as st,
            tc.tile_pool(name="xin", bufs=3) as xp,
            tc.tile_pool(name="qt", bufs=2) as qp,
            tc.tile_pool(name="expsim", bufs=6) as ep,
            tc.tile_pool(name="outut", bufs=2) as op_,
            tc.tile_pool(name="rcp", bufs=2) as rp,
            tc.tile_pool(name="bcast", bufs=3) as bp,
            tc.tile_pool(name="ysb", bufs=4) as yp,
        ):
            # ---- static loads, ordered by first consumer: ctx/wk/wv feed the
            # setup projections, wq + x chunk 0 feed the first Q projection;
            # wo/bo are issued inside the loop (first needed one chunk later).
            ctxs = st.tile([S, DCTX], f32, tag="ctxs")
            nc.sync.dma_start(ctxs[:], CTX[:])
            ident = st.tile([128, 128], f32, tag="ident")
            make_identity(nc, ident[:])
            wk = st.tile([128, KT, C], f32r, tag="wk")
            nc.sync.dma_start(wk[:], WK[:].bitcast(f32r).rearrange("(o p) i -> p o i", p=128))
            wv = st.tile([128, KT, C], f32r, tag="wv")
            nc.sync.dma_start(wv[:], WV[:].bitcast(f32r).rearrange("(o p) i -> p o i", p=128))
            wq = st.tile([128, CT, C], f32r, tag="wq")
            nc.sync.dma_start(wq[:], WQ[:].bitcast(f32r).rearrange("(o p) i -> p o i", p=128))
            wo = st.tile([128, CT, C], f32r, tag="wo")
            bo = st.tile([128, CT], f32, tag="bo")

            # selector weights (static), built with affine_select (iota
            # compare, out = compare(iota, 0) ? in_ : fill) in f32 scratch,
            # then CAST into f32r (matmul requires f32r-rounded producers).
            # sel77[j, h, c] = (c == h): rowsum of es_h lands on psum row h
            sel77f = st.tile([S, HEADS, HEADS], f32, tag="sel77f")
            nc.gpsimd.memset(sel77f[:], 0.0)
            nc.gpsimd.affine_select(
                out=sel77f[:], in_=sel77f[:],
                compare_op=mybir.AluOpType.not_equal, fill=1.0,
                base=0, channel_multiplier=0,
                pattern=[[-1, HEADS], [1, HEADS]])  # iota = c - h
            sel77 = st.tile([S, HEADS, HEADS], f32r, tag="sel77")
            nc.vector.tensor_copy(sel77[:], sel77f[:])
            # selbc[j, p, half, c] = (j == 2p + half): bc_p = bcast of rcp8 rows
            selbcf = st.tile([HEADS, NP, 2, DH], f32, tag="selbcf")
            nc.gpsimd.memset(selbcf[:], 0.0)
            nc.gpsimd.affine_select(
                out=selbcf[:], in_=selbcf[:],
                compare_op=mybir.AluOpType.not_equal, fill=1.0,
                base=0, channel_multiplier=1,
                pattern=[[-2, NP], [-1, 2], [0, DH]])  # iota = j - 2p - half
            selbc = st.tile([HEADS, NP, 2, DH], f32r, tag="selbc")
            nc.vector.tensor_copy(selbc[:], selbcf[:])

            # ---- setup: context transpose, K/V projections --------------------
            ctxT = st.tile([128, KT, S], f32r, tag="ctxT")
            ktp = st.tile([128, NP, S], f32r, tag="ktp")    # kT head-pairs
            # vpair[:, p, 0] = [v_2p | 0], vpair[:, p, 1] = [0 | v_2p+1]:
            # zero-padded M=128 stationaries so the AV pair accumulates into
            # one [128, TC] bank without col-tiling (quadrant 3 is invalid).
            vpair = st.tile([S, NP, 2, 128], f32r, tag="vpair")
            nc.gpsimd.memset(vpair[:].bitcast(f32), 0.0)
            with tc.tile_pool(name="ps_setup", bufs=1, space="PSUM") as ps_st:
                for ct in range(KT):
                    tp = ps_st.tile([128, S], f32, tag=f"ctx_t{ct % 2}")
                    nc.tensor.transpose(tp[:], ctxs[:, ct * 128:(ct + 1) * 128], ident[0:S, 0:S])
                    nc.vector.tensor_copy(ctxT[:, ct, :], tp[:])
                kps = ps_st.tile([S, C], f32, tag="kproj")
                vps = ps_st.tile([S, C], f32, tag="vproj")
                for ct in range(KT):
                    nc.tensor.matmul(kps[:], ctxT[:, ct, :], wk[:, ct, :],
                                     start=(ct == 0), stop=(ct == KT - 1))
                for ct in range(KT):
                    nc.tensor.matmul(vps[:], ctxT[:, ct, :], wv[:, ct, :],
                                     start=(ct == 0), stop=(ct == KT - 1))
                ksb = st.tile([S, C], f32, tag="ksb")
                nc.vector.tensor_copy(ksb[:], kps[:])
                for h in range(HEADS):
                    half = h % 2
                    nc.vector.tensor_copy(
                        vpair[:, h // 2, half, half * DH:half * DH + DH],
                        vps[:, h * DH:(h + 1) * DH])
                for h in range(HEADS):
                    tp = ps_st.tile([DH, S], f32, tag=f"k_t{h % 2}")
                    nc.tensor.transpose(tp[:], ksb[:, h * DH:(h + 1) * DH], ident[0:S, 0:S])
                    base = (h % 2) * DH
                    nc.vector.tensor_copy(ktp[base:base + DH, h // 2, :], tp[:])

            with (
                tc.tile_pool(name="ps_gemm", bufs=3, space="PSUM") as ps_g,
                tc.tile_pool(name="ps_sim", bufs=2, space="PSUM") as ps_sim,
                tc.tile_pool(name="ps_av", bufs=2, space="PSUM") as ps_av,
                tc.tile_pool(name="ps_rs", bufs=1, space="PSUM") as ps_rs,
            ):
                # ---- main loop over token chunks -----------------------------
                def oproj_group(t, ou, ct):
                    tsl = slice(t * TC, (t + 1) * TC)
                    py = ps_g.tile([128, TC], f32, tag="pg")
                    for it in range(CT):
                        nc.tensor.matmul(py[:], wo[:, it, ct * 128:(ct + 1) * 128], ou[:, it, :],
                                         start=(it == 0), stop=(it == CT - 1))
                    ys = yp.tile([128, TC], f32, tag="ys")
                    if ct % 2 == 0:
                        nc.scalar.activation(ys[:], py[:], AF.Identity, bias=bo[:, ct:ct + 1])
                    else:
                        nc.vector.tensor_scalar_add(ys[:], py[:], bo[:, ct:ct + 1])
                    nc.sync.dma_start(
                        Y[:].rearrange("(o p) t -> p o t", p=128)[:, ct, tsl], ys[:])

                prev = None
                for t in range(NT):
                    xs = xp.tile([128, CT, TC], f32r, tag="xs")
                    nc.sync.dma_start(
                        xs[:], X[:].bitcast(f32r).rearrange("(o p) t -> p o t", p=128)
                        [:, :, t * TC:(t + 1) * TC])
                    if t == 0:
                        # behind x chunk 0 on the queue: needed a chunk later
                        nc.sync.dma_start(
                            wo[:], WO[:].bitcast(f32r).rearrange("(o p) c -> p o c", p=128))
                        nc.sync.dma_start(bo[:], BO[:].rearrange("(o p) -> p o", p=128))

                    # Q projection -> qT [128, 4, TC] (i on partitions)
                    qt = qp.tile([128, CT, TC], f32r, tag="qt")
                    for it in range(CT):
                        pq = ps_g.tile([128, TC], f32, tag="pg")
                        for ct in range(CT):
                            nc.tensor.matmul(pq[:], wq[:, ct, it * 128:(it + 1) * 128], xs[:, ct, :],
                                             start=(ct == 0), stop=(ct == CT - 1))
                        nc.vector.tensor_copy(qt[:, it, :], pq[:])

                    if dbg and t == 0:
                        nc.sync.dma_start(DQ[:], qt[:].bitcast(f32))

                    # QK^T per head + exp (scale 1/8 fused in ACT); rowsums
                    # accumulate into one [8, TC] psum bank via sel77; AV pairs
                    # col-tiled into one [128, TC] bank; chunk t-1's O
                    # projection groups interleave as PE filler.
                    rs8 = ps_rs.tile([HEADS, TC], f32, tag="rs8")
                    exps = []
                    avps = []
                    ogroups = list(range(CT)) if prev is not None else []

                    def emit_oproj_filler():
                        if ogroups:
                            oproj_group(t - 1, prev[0], ogroups.pop(0))

                    for p in range(NP):
                        # QK pair back-to-back: row-tiled (bases 0/64), runs
                        # concurrently on the PE
                        for half in range(2):
                            h = 2 * p + half
                            base = half * DH
                            psim = ps_sim.tile([128, TC], f32, tag="psim")
                            nc.tensor.matmul(psim[0:S, :], ktp[base:base + DH, p, :],
                                             qt[base:base + DH, p, :])
                            es = ep.tile([S, TC], f32r, tag="exps")
                            nc.scalar.activation(es[:], psim[0:S, :], AF.Exp,
                                                 scale=DH ** -0.5)
                            exps.append(es)
                            if dbg and t == 0 and h == 0:
                                nc.sync.dma_start(DE[:], es[:].bitcast(f32))
                        pav = ps_av.tile([128, TC], f32, tag="pav")
                        avps.append(pav)
                        for half in range(2):
                            h = 2 * p + half
                            # rowsum accumulate + AV pair (zero-padded M=128
                            # stationaries accumulate into one psum bank)
                            nc.tensor.matmul(rs8[:], sel77[:, h, :], exps[h][:],
                                             start=(h == 0), stop=(h == HEADS - 1))
                            nc.tensor.matmul(pav[:], vpair[:, p, half, :],
                                             exps[h][:], start=(half == 0),
                                             stop=(half == 1))
                        emit_oproj_filler()

                    # recip of packed rowsums (single DVE op, ~18-bit accurate)
                    rcp8f = rp.tile([HEADS, TC], f32, tag="rcp8f")
                    nc.vector.reciprocal_approx_fast(rcp8f[:], rs8[:])
                    rcp8 = rp.tile([HEADS, TC], f32r, tag="rcp8")
                    nc.vector.tensor_copy(rcp8[:], rcp8f[:])
                    if dbg and t == 0:
                        nc.sync.dma_start(DRS[:], rs8[:])
                        nc.sync.dma_start(DRC[:], rcp8[:])

                    # broadcast + fused evac/normalize per pair:
                    # bc_p[c,t] = rcp8[2p + c//64, t]; ou[:,p,:] = pav_p * bc_p
                    ou = op_.tile([128, CT, TC], f32r, tag="ou")
                    for p in range(NP):
                        pbc = ps_sim.tile([128, TC], f32, tag="psim")
                        nc.tensor.matmul(pbc[:], selbc[:, p, :, :],
                                         rcp8[:].bitcast(f32r))
                        bcs = bp.tile([128, TC], f32, tag="bcs")
                        nc.scalar.activation(bcs[:], pbc[:], AF.Copy)
                        nc.vector.tensor_tensor(
                            ou[:, p, :], avps[p][:], bcs[:],
                            mybir.AluOpType.mult)
                        if dbg and t == 0 and p == 0:
                            nc.sync.dma_start(DBC[:], bcs[:])
                    if dbg and t == 0:
                        nc.sync.dma_start(DOU[:], ou[:].bitcast(f32))

                    # leftover O-projection groups for chunk t-1
                    while ogroups:
                        emit_oproj_filler()

                    prev = (ou,)

                # drain: O projection of the last chunk
                for ct in range(CT):
                    oproj_group(NT - 1, prev[0], ct)

    nc.compile()
    return nc


def _get_nc():
    global _BUILT
    if _BUILT is None:
        _BUILT = _build()
    return _BUILT


def kernel(x, context, Wq, Wk, Wv, Wo, bo):
    from concourse.bass_utils import run_bass_kernel_spmd

    B = x.shape[0]
    assert B == 8 and x.shape == (8, C, 64, 64)
    nc = _get_nc()
    x = np.ascontiguousarray(np.asarray(x, dtype=np.float32))
    in_maps = [
        {
            "x": x[b].reshape(C, T),
            "ctx": np.ascontiguousarray(np.asarray(context[b], np.float32)),
            "wq": np.asarray(Wq, np.float32),
            "wk": np.asarray(Wk, np.float32),
            "wv": np.asarray(Wv, np.float32),
            "wo": np.asarray(Wo, np.float32),
            "bo": np.asarray(bo, np.float32),
        }
        for b in range(B)
    ]
    res = run_bass_kernel_spmd(nc, in_maps, core_ids=list(range(8)))
    return np.stack([r["y"].reshape(C, 64, 64) for r in res.results]).astype(np.float32)
